# revision 17
# baseline (speedup 1.0000x reference)
"""Trainium2 Bass kernel for a 2-layer tanh RNN + final Linear.

Reference computation (B=20, T=2048, I=256, H=512, O=128):
    pre0 = x @ W_ih0.T + b_ih0
    h_t  = tanh(pre0_t + h_{t-1} @ W_hh0.T + b_hh0)        (layer 0)
    pre1 = out0 @ W_ih1.T + b_ih1
    g_t  = tanh(pre1_t + g_{t-1} @ W_hh1.T + b_hh1)        (layer 1)
    out  = g_{T-1} @ W_fc.T + b_fc                          -> [B, O]

Strategy (8 cores, data-parallel over batch, 3 rows/core after padding
20->24):
  * Everything on-chip is kept feature-major ("transposed"): state is
    hT [128 part, 4 chunks, B_local], so the weights-stationary matmul
    (lhsT = 128x128 W.T block, moving = tiny [128, B_local] state)
    produces the next state directly in transposed layout -- no per-step
    transpose.
  * All matmul operands are fp16 (fp32 PSUM accumulate): fp16 weights
    get the fast-weight-load path, which is what bounds the sequential
    recurrence (16 LDW+MM per step).
  * The input projection for step t is pre-copied into the PSUM
    accumulator by ScalarE; the 16 recurrence matmuls then accumulate
    onto it via start=False (has_written bits stay set after the step-0
    identity-matmul prime), so no separate DVE add sits on the critical
    chain.
  * Input projections / transposes are token-parallel and run before
    each recurrence.
"""

import sys
from contextlib import ExitStack

import numpy as np

sys.path.insert(0, "/opt/trn_rl_repo")

import concourse.bass as bass
import concourse.bacc as bacc
import concourse.tile as tile
from concourse import mybir
from concourse.bass_utils import run_bass_kernel_spmd

F32 = mybir.dt.float32
F16 = mybir.dt.float16
AFT = mybir.ActivationFunctionType

B, T, I, H, O = 20, 2048, 256, 512, 128
NCORES = 8
BPAD = 24
BL = BPAD // NCORES  # batch rows per core
HC = H // 128  # 4 hidden chunks
IC = I // 128  # 2 input chunks

# Pipelined recurrence: two PSUM banks (half A = hidden chunks 0-1, half
# B = chunks 2-3), each primed every step with an identity matmul whose
# moving operand is that half's input projection (start=True also clears
# the bank's has_written bits, so the 8 recurrence matmuls accumulate
# onto pre via start=False). Matmul groups are ordered so the tanh of
# half A overlaps the matmuls feeding half B, and the next step's
# matmuls only wait on the half they actually read.
PIPE = True


def _recurrence(nc, rps_pool, whh_sb, ident_sb, pre_jc, out_writer,
                full_state, t_steps, zeros_sb):
    """Emit one RNN layer's sequential recurrence.

    whh_sb:      SBUF weights [128, HC(kc), HC(jc), 128] f16
    pre_jc:      (t, j0, j1) -> AP [128, j1-j0, BL] fp16 input projection
    out_writer:  (t, j0, j1) -> AP [128, j1-j0, BL] fp16 tanh destination
    full_state:  t -> AP [128, HC, BL] (the state written at step t)
    """
    if not PIPE:
        rps = rps_pool.tile([128, HC, BL], F32, tag="rps")
        nc.tensor.matmul(rps[:, :, :], ident_sb[:], pre_jc(0, 0, HC),
                         start=True, stop=True)
        for t in range(t_steps):
            prev = zeros_sb if t == 0 else full_state(t - 1)
            for jc in range(HC):
                for kc in range(HC):
                    nc.tensor.matmul(
                        rps[:, jc, :], whh_sb[:, kc, jc, :], prev[:, kc, :],
                        start=False, stop=kc == HC - 1)
            nc.scalar.activation(out_writer(t, 0, HC), rps[:, :, :], AFT.Tanh)
            if t + 1 < t_steps:
                nc.scalar.copy(rps[:, :, :], pre_jc(t + 1, 0, HC))
        return

    rpsA = rps_pool.tile([128, 2, BL], F32, tag="rpsA")
    rpsB = rps_pool.tile([128, 2, BL], F32, tag="rpsB")

    def mm(rps, jc, kc, prev, stop):
        nc.tensor.matmul(rps[:, jc % 2, :], whh_sb[:, kc, jc, :],
                         prev[:, kc, :], start=False, stop=stop)

    for t in range(t_steps):
        prev = zeros_sb if t == 0 else full_state(t - 1)
        # prime A with pre_t[0:2]; G1 = A-half rows x first-half k
        nc.tensor.matmul(rpsA[:, :, :], ident_sb[:], pre_jc(t, 0, 2),
                         start=True, stop=True)
        for jc in (0, 1):
            for kc in (0, 1):
                mm(rpsA, jc, kc, prev, False)
        nc.tensor.matmul(rpsB[:, :, :], ident_sb[:], pre_jc(t, 2, 4),
                         start=True, stop=True)
        for jc in (2, 3):
            for kc in (0, 1):
                mm(rpsB, jc, kc, prev, False)
        for jc in (0, 1):
            for kc in (2, 3):
                mm(rpsA, jc, kc, prev, kc == 3)
        nc.scalar.activation(out_writer(t, 0, 2), rpsA[:, :, :], AFT.Tanh)
        for jc in (2, 3):
            for kc in (2, 3):
                mm(rpsB, jc, kc, prev, kc == 3)
        nc.scalar.activation(out_writer(t, 2, 4), rpsB[:, :, :], AFT.Tanh)


def _build(t_steps=T, bl=BL):
    nc = bacc.Bacc("TRN2", target_bir_lowering=False, debug=False,
                   num_devices=NCORES)

    x_d = nc.dram_tensor("x", [bl, t_steps, I], F32, kind="ExternalInput").ap()
    wih0_d = nc.dram_tensor("wih0", [128, IC, HC, 128], F16, kind="ExternalInput").ap()
    whh0_d = nc.dram_tensor("whh0", [128, HC, HC, 128], F16, kind="ExternalInput").ap()
    wih1_d = nc.dram_tensor("wih1", [128, HC, HC, 128], F16, kind="ExternalInput").ap()
    whh1_d = nc.dram_tensor("whh1", [128, HC, HC, 128], F16, kind="ExternalInput").ap()
    wfc_d = nc.dram_tensor("wfc", [128, HC, 128], F16, kind="ExternalInput").ap()
    b0_d = nc.dram_tensor("b0", [128, HC], F32, kind="ExternalInput").ap()
    b1_d = nc.dram_tensor("b1", [128, HC], F32, kind="ExternalInput").ap()
    bfc_d = nc.dram_tensor("bfc", [128, 1], F32, kind="ExternalInput").ap()
    ident_d = nc.dram_tensor("ident", [128, 128], F16, kind="ExternalInput").ap()
    outT_d = nc.dram_tensor("outT", [128, bl], F32, kind="ExternalOutput").ap()

    with tile.TileContext(nc) as tc, ExitStack() as ctx:
        const = ctx.enter_context(tc.tile_pool(name="const", bufs=1))

        def _load_const(shape, dtype, src, tag):
            t_ = const.tile(shape, dtype, tag=tag)
            nc.sync.dma_start(t_[:], src)
            return t_

        wih0 = _load_const([128, IC, HC, 128], F16, wih0_d, "wih0")
        whh0 = _load_const([128, HC, HC, 128], F16, whh0_d, "whh0")
        wih1 = _load_const([128, HC, HC, 128], F16, wih1_d, "wih1")
        whh1 = _load_const([128, HC, HC, 128], F16, whh1_d, "whh1")
        wfc = _load_const([128, HC, 128], F16, wfc_d, "wfc")
        b0 = _load_const([128, HC], F32, b0_d, "b0")
        b1 = _load_const([128, HC], F32, b1_d, "b1")
        bfc = _load_const([128, 1], F32, bfc_d, "bfc")
        ident = _load_const([128, 128], F16, ident_d, "ident")
        zeros = const.tile([128, HC, BL], F16, tag="zeros")
        nc.vector.memset(zeros[:], 0.0)

        out0_pool = ctx.enter_context(tc.tile_pool(name="out0", bufs=1))
        out0 = out0_pool.tile([128, HC, bl, t_steps], F16)

        pps = ctx.enter_context(tc.tile_pool(name="pps", bufs=2, space="PSUM"))
        rps_pool = ctx.enter_context(tc.tile_pool(name="rps", bufs=1, space="PSUM"))
        dve_pool = ctx.enter_context(tc.tile_pool(name="dvetmp", bufs=2))
        h1_pool = ctx.enter_context(tc.tile_pool(name="h1ring", bufs=3))

        with ExitStack() as s01:
            big01 = s01.enter_context(tc.tile_pool(name="big01", bufs=1))
            xT = big01.tile([128, IC, bl * t_steps], F16)
            pre0 = big01.tile([128, HC, bl, t_steps], F16)

            # ---- Stage 1: load + cast + transpose x -> xT -------------
            with ExitStack() as s1:
                xload = s1.enter_context(tc.tile_pool(name="xload", bufs=3))
                xcast = s1.enter_context(tc.tile_pool(name="xcast", bufs=3))
                tps = s1.enter_context(tc.tile_pool(name="tps", bufs=2, space="PSUM"))
                for b in range(bl):
                    for tq in range(t_steps // 128):
                        xt = xload.tile([128, I], F32, tag="xt")
                        nc.sync.dma_start(xt[:], x_d[b, tq * 128:(tq + 1) * 128, :])
                        xc = xcast.tile([128, I], F16, tag="xc")
                        nc.vector.tensor_copy(xc[:], xt[:])
                        for ic in range(IC):
                            tp = tps.tile([128, 128], F16, tag="tp")
                            nc.tensor.transpose(
                                tp[:], xc[:, ic * 128:(ic + 1) * 128], ident[:])
                            base = b * t_steps + tq * 128
                            nc.vector.tensor_copy(
                                xT[:, ic, base:base + 128], tp[:])

            # ---- Stage 2: pre0 = W_ih0 @ xT + bias0 (fp16) ------------
            twid = min(512, t_steps)
            tc4n = t_steps // twid
            for b in range(bl):
                for t4 in range(tc4n):
                    for jc in range(HC):
                        ps = pps.tile([128, 512], F32, tag="pps")
                        for ic in range(IC):
                            nc.tensor.matmul(
                                ps[:, :twid],
                                wih0[:, ic, jc, :],
                                xT[:, ic, b * t_steps + t4 * twid:
                                   b * t_steps + (t4 + 1) * twid],
                                start=ic == 0, stop=ic == IC - 1,
                            )
                        nc.scalar.add(
                            pre0[:, jc, b, t4 * twid:(t4 + 1) * twid],
                            ps[:, :twid], b0[:, jc:jc + 1])

            # ---- Stage 3: recurrence layer 0 --------------------------
            _recurrence(
                nc, rps_pool, whh0, ident,
                lambda t, j0, j1: pre0[:, j0:j1, :, t],
                lambda t, j0, j1: out0[:, j0:j1, :, t],
                lambda t: out0[:, :, :, t],
                t_steps, zeros)

        # big01 released: xT + pre0 space is free for pre1.
        with ExitStack() as s45:
            big45 = s45.enter_context(tc.tile_pool(name="big45", bufs=1))
            pre1 = big45.tile([128, HC, bl, t_steps], F16)

            # ---- Stage 4: pre1 = W_ih1 @ out0 + bias1 -----------------
            for b in range(bl):
                for t4 in range(tc4n):
                    for jc in range(HC):
                        ps = pps.tile([128, 512], F32, tag="pps")
                        for kc in range(HC):
                            nc.tensor.matmul(
                                ps[:, :twid],
                                wih1[:, kc, jc, :],
                                out0[:, kc, b, t4 * twid:(t4 + 1) * twid],
                                start=kc == 0, stop=kc == HC - 1,
                            )
                        nc.scalar.add(
                            pre1[:, jc, b, t4 * twid:(t4 + 1) * twid],
                            ps[:, :twid], b1[:, jc:jc + 1])

            # ---- Stage 5: recurrence layer 1 --------------------------
            h1_tiles = {}

            def h1_tile(t):
                if t not in h1_tiles:
                    h1_tiles[t] = h1_pool.tile([128, HC, BL], F16, tag="h1",
                                               name=f"h1_{t}")
                return h1_tiles[t]

            _recurrence(
                nc, rps_pool, whh1, ident,
                lambda t, j0, j1: pre1[:, j0:j1, :, t],
                lambda t, j0, j1: h1_tile(t)[:, j0:j1, :],
                lambda t: h1_tile(t)[:, :, :],
                t_steps, zeros)

            # ---- Stage 6: FC on last timestep -------------------------
            h_last = h1_tile(t_steps - 1)
            fps = pps.tile([128, 512], F32, tag="pps")
            for kc in range(HC):
                nc.tensor.matmul(
                    fps[:, :BL], wfc[:, kc, :], h_last[:, kc, :],
                    start=kc == 0, stop=kc == HC - 1)
            fc_sb = dve_pool.tile([128, BL], F32, tag="fc")
            nc.scalar.add(fc_sb[:], fps[:, :BL], bfc[:])
            nc.sync.dma_start(outT_d, fc_sb[:])

    nc.compile()
    return nc


def _pack_w(w):
    """[out, in] fp32 -> [128, in_chunks, out_chunks, 128] fp16 block layout."""
    o, i = w.shape
    icn, ocn = i // 128, o // 128
    return np.ascontiguousarray(
        w.T.reshape(icn, 128, ocn, 128).transpose(1, 0, 2, 3)).astype(np.float16)


def _make_in_maps(inputs):
    x = np.asarray(inputs["x"], np.float32)
    consts = {
        "wih0": _pack_w(np.asarray(inputs["W_ih0"], np.float32)),
        "whh0": _pack_w(np.asarray(inputs["W_hh0"], np.float32)),
        "wih1": _pack_w(np.asarray(inputs["W_ih1"], np.float32)),
        "whh1": _pack_w(np.asarray(inputs["W_hh1"], np.float32)),
        "wfc": _pack_w(np.asarray(inputs["W_fc"], np.float32))
        .reshape(128, HC, 128),
        "b0": np.ascontiguousarray(
            (np.asarray(inputs["b_ih0"]) + np.asarray(inputs["b_hh0"]))
            .reshape(HC, 128).T).astype(np.float32),
        "b1": np.ascontiguousarray(
            (np.asarray(inputs["b_ih1"]) + np.asarray(inputs["b_hh1"]))
            .reshape(HC, 128).T).astype(np.float32),
        "bfc": np.asarray(inputs["b_fc"], np.float32).reshape(128, 1),
        "ident": np.eye(128, dtype=np.float16),
    }
    xp = np.zeros((BPAD, T, I), np.float32)
    xp[:B] = x
    return [
        {"x": np.ascontiguousarray(xp[c * BL:(c + 1) * BL]), **consts}
        for c in range(NCORES)
    ]


def _gather(results):
    out = np.zeros((BPAD, O), np.float32)
    for c in range(NCORES):
        out[c * BL:(c + 1) * BL] = results[c]["outT"].T
    return out[:B]


def kernel(x, h0, W_ih0, W_hh0, b_ih0, b_hh0, W_ih1, W_hh1, b_ih1, b_hh1,
           W_fc, b_fc):
    inputs = dict(x=x, W_ih0=W_ih0, W_hh0=W_hh0, b_ih0=b_ih0, b_hh0=b_hh0,
                  W_ih1=W_ih1, W_hh1=W_hh1, b_ih1=b_ih1, b_hh1=b_hh1,
                  W_fc=W_fc, b_fc=b_fc)
    nc = _build()
    in_maps = _make_in_maps(inputs)
    res = run_bass_kernel_spmd(nc, in_maps, list(range(NCORES)))
    return _gather(res.results)


if __name__ == "__main__":
    ins = {k: np.asarray(v) for k, v in np.load(sys.argv[1]).items()} \
        if len(sys.argv) > 1 else None


# revision 21
# speedup vs baseline: 51.4322x; 51.4322x over previous
"""Trainium2 Bass kernel for a 2-layer tanh RNN + final Linear.

Reference computation (B=20, T=2048, I=256, H=512, O=128):
    pre0 = x @ W_ih0.T + b_ih0
    h_t  = tanh(pre0_t + h_{t-1} @ W_hh0.T + b_hh0)        (layer 0)
    pre1 = out0 @ W_ih1.T + b_ih1
    g_t  = tanh(pre1_t + g_{t-1} @ W_hh1.T + b_hh1)        (layer 1)
    out  = g_{T-1} @ W_fc.T + b_fc                          -> [B, O]

Strategy (8 cores, data-parallel over batch, 3 rows/core after padding
20->24):
  * Everything on-chip is kept feature-major ("transposed"): state is
    hT [128 part, 4 chunks, B_local], so the weights-stationary matmul
    (lhsT = 128x128 W.T block, moving = tiny [128, B_local] state)
    produces the next state directly in transposed layout -- no per-step
    transpose.
  * All matmul operands are fp16 (fp32 PSUM accumulate): fp16 weights
    get the fast-weight-load path, which is what bounds the sequential
    recurrence (16 LDW+MM per step).
  * The input projection for step t is written into the PSUM accumulator
    by an identity matmul with start=True (clears the bank's has_written
    bits); the 16 recurrence matmuls accumulate onto it via start=False,
    so no add op sits on the critical chain. The accumulator is split
    across two PSUM banks (hidden chunks 0-1 / 2-3) and the matmul
    groups are ordered so each half's tanh overlaps the other half's
    matmuls.
  * Input projections / transposes are token-parallel and run before
    each recurrence.
"""

import sys
from contextlib import ExitStack

import numpy as np

sys.path.insert(0, "/opt/trn_rl_repo")

import concourse.bass as bass
import concourse.bacc as bacc
import concourse.tile as tile
from concourse import mybir
from concourse.bass_utils import run_bass_kernel_spmd

F32 = mybir.dt.float32
F16 = mybir.dt.float16
AFT = mybir.ActivationFunctionType

B, T, I, H, O = 20, 2048, 256, 512, 128
NCORES = 8
BPAD = 24
BL = BPAD // NCORES  # batch rows per core
HC = H // 128  # 4 hidden chunks
IC = I // 128  # 2 input chunks

# Pipelined recurrence: two PSUM banks (half A = hidden chunks 0-1, half
# B = chunks 2-3), each primed every step with an identity matmul whose
# moving operand is that half's input projection (start=True also clears
# the bank's has_written bits, so the 8 recurrence matmuls accumulate
# onto pre via start=False). Matmul groups are ordered so the tanh of
# half A overlaps the matmuls feeding half B, and the next step's
# matmuls only wait on the half they actually read.
PIPE = True


def _recurrence(nc, rps_pool, whh_sb, ident_sb, pre_jc, out_writer,
                full_state, t_steps, zeros_sb):
    """Emit one RNN layer's sequential recurrence.

    whh_sb:      SBUF weights [128, HC(kc), HC(jc), 128] f16
    pre_jc:      (t, j0, j1) -> AP [128, j1-j0, BL] fp16 input projection
    out_writer:  (t, j0, j1) -> AP [128, j1-j0, BL] fp16 tanh destination
    full_state:  t -> AP [128, HC, BL] (the state written at step t)
    """
    if not PIPE:
        rps = rps_pool.tile([128, HC, BL], F32, tag="rps")
        nc.tensor.matmul(rps[:, :, :], ident_sb[:], pre_jc(0, 0, HC),
                         start=True, stop=True)
        for t in range(t_steps):
            prev = zeros_sb if t == 0 else full_state(t - 1)
            for jc in range(HC):
                for kc in range(HC):
                    nc.tensor.matmul(
                        rps[:, jc, :], whh_sb[:, kc, jc, :], prev[:, kc, :],
                        start=False, stop=kc == HC - 1)
            nc.scalar.activation(out_writer(t, 0, HC), rps[:, :, :], AFT.Tanh)
            if t + 1 < t_steps:
                nc.scalar.copy(rps[:, :, :], pre_jc(t + 1, 0, HC))
        return

    rpsA = rps_pool.tile([128, 2, BL], F32, tag="rpsA")
    rpsB = rps_pool.tile([128, 2, BL], F32, tag="rpsB")

    def mm(rps, jc, kc, prev, stop):
        nc.tensor.matmul(rps[:, jc % 2, :], whh_sb[:, kc, jc, :],
                         prev[:, kc, :], start=False, stop=stop)

    for t in range(t_steps):
        prev = zeros_sb if t == 0 else full_state(t - 1)
        # Prime each bank with this step's input projection: start=True
        # clears the bank's has_written bits and writes pre_t, so the 8
        # recurrence matmuls into that bank accumulate onto it.
        nc.tensor.matmul(rpsA[:, :, :], ident_sb[:], pre_jc(t, 0, 2),
                         start=True, stop=True)
        for jc in (0, 1):
            for kc in (0, 1):
                mm(rpsA, jc, kc, prev, False)
        nc.tensor.matmul(rpsB[:, :, :], ident_sb[:], pre_jc(t, 2, 4),
                         start=True, stop=True)
        for jc in (2, 3):
            for kc in (0, 1):
                mm(rpsB, jc, kc, prev, False)
        for jc in (0, 1):
            for kc in (2, 3):
                mm(rpsA, jc, kc, prev, kc == 3)
        nc.scalar.activation(out_writer(t, 0, 2), rpsA[:, :, :], AFT.Tanh)
        for jc in (2, 3):
            for kc in (2, 3):
                mm(rpsB, jc, kc, prev, kc == 3)
        nc.scalar.activation(out_writer(t, 2, 4), rpsB[:, :, :], AFT.Tanh)


def _build(t_steps=T, bl=BL):
    nc = bacc.Bacc("TRN2", target_bir_lowering=False, debug=False,
                   num_devices=NCORES)

    x_d = nc.dram_tensor("x", [bl, t_steps, I], F32, kind="ExternalInput").ap()
    wih0_d = nc.dram_tensor("wih0", [128, IC, HC, 128], F16, kind="ExternalInput").ap()
    whh0_d = nc.dram_tensor("whh0", [128, HC, HC, 128], F16, kind="ExternalInput").ap()
    wih1_d = nc.dram_tensor("wih1", [128, HC, HC, 128], F16, kind="ExternalInput").ap()
    whh1_d = nc.dram_tensor("whh1", [128, HC, HC, 128], F16, kind="ExternalInput").ap()
    wfc_d = nc.dram_tensor("wfc", [128, HC, 128], F16, kind="ExternalInput").ap()
    b0_d = nc.dram_tensor("b0", [128, HC], F32, kind="ExternalInput").ap()
    b1_d = nc.dram_tensor("b1", [128, HC], F32, kind="ExternalInput").ap()
    bfc_d = nc.dram_tensor("bfc", [128, 1], F32, kind="ExternalInput").ap()
    ident_d = nc.dram_tensor("ident", [128, 128], F16, kind="ExternalInput").ap()
    outT_d = nc.dram_tensor("outT", [128, bl], F32, kind="ExternalOutput").ap()

    with tile.TileContext(nc) as tc, ExitStack() as ctx:
        const = ctx.enter_context(tc.tile_pool(name="const", bufs=1))

        def _load_const(shape, dtype, src, tag):
            t_ = const.tile(shape, dtype, tag=tag)
            nc.sync.dma_start(t_[:], src)
            return t_

        wih0 = _load_const([128, IC, HC, 128], F16, wih0_d, "wih0")
        whh0 = _load_const([128, HC, HC, 128], F16, whh0_d, "whh0")
        wih1 = _load_const([128, HC, HC, 128], F16, wih1_d, "wih1")
        whh1 = _load_const([128, HC, HC, 128], F16, whh1_d, "whh1")
        wfc = _load_const([128, HC, 128], F16, wfc_d, "wfc")
        b0 = _load_const([128, HC], F32, b0_d, "b0")
        b1 = _load_const([128, HC], F32, b1_d, "b1")
        bfc = _load_const([128, 1], F32, bfc_d, "bfc")
        ident = _load_const([128, 128], F16, ident_d, "ident")
        zeros = const.tile([128, HC, BL], F16, tag="zeros")
        nc.vector.memset(zeros[:], 0.0)

        out0_pool = ctx.enter_context(tc.tile_pool(name="out0", bufs=1))
        out0 = out0_pool.tile([128, HC, bl, t_steps], F16)

        pps = ctx.enter_context(tc.tile_pool(name="pps", bufs=2, space="PSUM"))
        rps_pool = ctx.enter_context(tc.tile_pool(name="rps", bufs=1, space="PSUM"))
        dve_pool = ctx.enter_context(tc.tile_pool(name="dvetmp", bufs=2))
        h1_pool = ctx.enter_context(tc.tile_pool(name="h1ring", bufs=3))

        with ExitStack() as s01:
            big01 = s01.enter_context(tc.tile_pool(name="big01", bufs=1))
            xT = big01.tile([128, IC, bl * t_steps], F16)
            pre0 = big01.tile([128, HC, bl, t_steps], F16)

            # ---- Stage 1: load + cast + transpose x -> xT -------------
            with ExitStack() as s1:
                xload = s1.enter_context(tc.tile_pool(name="xload", bufs=3))
                xcast = s1.enter_context(tc.tile_pool(name="xcast", bufs=3))
                tps = s1.enter_context(tc.tile_pool(name="tps", bufs=2, space="PSUM"))
                for b in range(bl):
                    for tq in range(t_steps // 128):
                        xt = xload.tile([128, I], F32, tag="xt")
                        nc.sync.dma_start(xt[:], x_d[b, tq * 128:(tq + 1) * 128, :])
                        xc = xcast.tile([128, I], F16, tag="xc")
                        nc.vector.tensor_copy(xc[:], xt[:])
                        for ic in range(IC):
                            tp = tps.tile([128, 128], F16, tag="tp")
                            nc.tensor.transpose(
                                tp[:], xc[:, ic * 128:(ic + 1) * 128], ident[:])
                            base = b * t_steps + tq * 128
                            nc.vector.tensor_copy(
                                xT[:, ic, base:base + 128], tp[:])

            # ---- Stage 2: pre0 = W_ih0 @ xT + bias0 (fp16) ------------
            twid = min(512, t_steps)
            tc4n = t_steps // twid
            for b in range(bl):
                for t4 in range(tc4n):
                    for jc in range(HC):
                        ps = pps.tile([128, 512], F32, tag="pps")
                        for ic in range(IC):
                            nc.tensor.matmul(
                                ps[:, :twid],
                                wih0[:, ic, jc, :],
                                xT[:, ic, b * t_steps + t4 * twid:
                                   b * t_steps + (t4 + 1) * twid],
                                start=ic == 0, stop=ic == IC - 1,
                            )
                        nc.scalar.add(
                            pre0[:, jc, b, t4 * twid:(t4 + 1) * twid],
                            ps[:, :twid], b0[:, jc:jc + 1])

            # ---- Stage 3: recurrence layer 0 --------------------------
            _recurrence(
                nc, rps_pool, whh0, ident,
                lambda t, j0, j1: pre0[:, j0:j1, :, t],
                lambda t, j0, j1: out0[:, j0:j1, :, t],
                lambda t: out0[:, :, :, t],
                t_steps, zeros)

        # big01 released: xT + pre0 space is free for pre1.
        with ExitStack() as s45:
            big45 = s45.enter_context(tc.tile_pool(name="big45", bufs=1))
            pre1 = big45.tile([128, HC, bl, t_steps], F16)

            # ---- Stage 4: pre1 = W_ih1 @ out0 + bias1 -----------------
            for b in range(bl):
                for t4 in range(tc4n):
                    for jc in range(HC):
                        ps = pps.tile([128, 512], F32, tag="pps")
                        for kc in range(HC):
                            nc.tensor.matmul(
                                ps[:, :twid],
                                wih1[:, kc, jc, :],
                                out0[:, kc, b, t4 * twid:(t4 + 1) * twid],
                                start=kc == 0, stop=kc == HC - 1,
                            )
                        nc.scalar.add(
                            pre1[:, jc, b, t4 * twid:(t4 + 1) * twid],
                            ps[:, :twid], b1[:, jc:jc + 1])

            # ---- Stage 5: recurrence layer 1 --------------------------
            h1_tiles = {}

            def h1_tile(t):
                if t not in h1_tiles:
                    h1_tiles[t] = h1_pool.tile([128, HC, BL], F16, tag="h1",
                                               name=f"h1_{t}")
                return h1_tiles[t]

            _recurrence(
                nc, rps_pool, whh1, ident,
                lambda t, j0, j1: pre1[:, j0:j1, :, t],
                lambda t, j0, j1: h1_tile(t)[:, j0:j1, :],
                lambda t: h1_tile(t)[:, :, :],
                t_steps, zeros)

            # ---- Stage 6: FC on last timestep -------------------------
            h_last = h1_tile(t_steps - 1)
            fps = pps.tile([128, 512], F32, tag="pps")
            for kc in range(HC):
                nc.tensor.matmul(
                    fps[:, :BL], wfc[:, kc, :], h_last[:, kc, :],
                    start=kc == 0, stop=kc == HC - 1)
            fc_sb = dve_pool.tile([128, BL], F32, tag="fc")
            nc.scalar.add(fc_sb[:], fps[:, :BL], bfc[:])
            nc.sync.dma_start(outT_d, fc_sb[:])

    nc.compile()
    return nc


def _pack_w(w):
    """[out, in] fp32 -> [128, in_chunks, out_chunks, 128] fp16 block layout."""
    o, i = w.shape
    icn, ocn = i // 128, o // 128
    return np.ascontiguousarray(
        w.T.reshape(icn, 128, ocn, 128).transpose(1, 0, 2, 3)).astype(np.float16)


def _make_in_maps(inputs):
    x = np.asarray(inputs["x"], np.float32)
    consts = {
        "wih0": _pack_w(np.asarray(inputs["W_ih0"], np.float32)),
        "whh0": _pack_w(np.asarray(inputs["W_hh0"], np.float32)),
        "wih1": _pack_w(np.asarray(inputs["W_ih1"], np.float32)),
        "whh1": _pack_w(np.asarray(inputs["W_hh1"], np.float32)),
        "wfc": _pack_w(np.asarray(inputs["W_fc"], np.float32))
        .reshape(128, HC, 128),
        "b0": np.ascontiguousarray(
            (np.asarray(inputs["b_ih0"]) + np.asarray(inputs["b_hh0"]))
            .reshape(HC, 128).T).astype(np.float32),
        "b1": np.ascontiguousarray(
            (np.asarray(inputs["b_ih1"]) + np.asarray(inputs["b_hh1"]))
            .reshape(HC, 128).T).astype(np.float32),
        "bfc": np.asarray(inputs["b_fc"], np.float32).reshape(128, 1),
        "ident": np.eye(128, dtype=np.float16),
    }
    xp = np.zeros((BPAD, T, I), np.float32)
    xp[:B] = x
    return [
        {"x": np.ascontiguousarray(xp[c * BL:(c + 1) * BL]), **consts}
        for c in range(NCORES)
    ]


def _gather(results):
    out = np.zeros((BPAD, O), np.float32)
    for c in range(NCORES):
        out[c * BL:(c + 1) * BL] = results[c]["outT"].T
    return out[:B]


def kernel(x, h0, W_ih0, W_hh0, b_ih0, b_hh0, W_ih1, W_hh1, b_ih1, b_hh1,
           W_fc, b_fc):
    inputs = dict(x=x, W_ih0=W_ih0, W_hh0=W_hh0, b_ih0=b_ih0, b_hh0=b_hh0,
                  W_ih1=W_ih1, W_hh1=W_hh1, b_ih1=b_ih1, b_hh1=b_hh1,
                  W_fc=W_fc, b_fc=b_fc)
    nc = _build()
    in_maps = _make_in_maps(inputs)
    res = run_bass_kernel_spmd(nc, in_maps, list(range(NCORES)))
    return _gather(res.results)


if __name__ == "__main__":
    ins = {k: np.asarray(v) for k, v in np.load(sys.argv[1]).items()} \
        if len(sys.argv) > 1 else None
